# revision 18
# baseline (speedup 1.0000x reference)
"""Spatial multi-head attention kernel for Trainium2 (8 NeuronCores).

Problem: B=8, T=64, N=170 nodes, C=128 channels, H=8 heads, D=16.
Attention over nodes N, independent per (b, t, h).

Strategy (v2):
- Pure data-parallel over B: core b computes batch b fully (no collectives).
- Host transposes inputs to channel-major [B, T, C, N] bf16; output returned
  channel-major [T, C, N] f32 per core and transposed back on host.
- Interleaved head layout: head h lives at partition rows
  32*(h%4) + 16*(h//4) + d.  One q projection (wq holds all 8 heads);
  k is projected twice (wka: heads 0-3 with heads 4-7 columns zeroed, wkb:
  the complement) so the K=32 energy matmuls contract only the right head.
- Energy per timestep packed dense into 4 PSUM banks as two 4-head halves;
  each half exp'd in a single ACT instruction ([128, 2, 3, 170] view).
  Only the Exp table set is ever loaded (reciprocal is done on DVE).
- attn@v via col-tiled M=17 matmuls with a ones column appended to v
  (denominator lands in partition 32j+16 of the av bank).
- Denominators for 8 timesteps gathered by matmul (G_i, accumulating) into
  one PSUM bank, reciprocal'd once per group with DVE
  reciprocal_approx_fast, then expanded per timestep with a selector
  matmul; normalization is one DVE multiply per timestep.
- Projections / output projection / DMA batched over 2-timestep chunks.
"""
import sys

sys.path.insert(0, "/opt/trn_rl_repo")

import numpy as np

import concourse.bacc as bacc
import concourse.mybir as mybir
import concourse.tile as tile
from concourse.bass_utils import run_bass_kernel_spmd

B, T, N, C = 8, 64, 170, 128
H, D = 8, 16
NC0 = 128            # first key chunk
NC1 = N - NC0        # 42
TC = 2               # timesteps per proj/io chunk
DG = 8               # timesteps per denominator group
F32 = np.float32
DT = mybir.dt.float32
BF = mybir.dt.bfloat16
EXP = mybir.ActivationFunctionType.Exp

_cache = {}


def _build_kernel():
    nc = bacc.Bacc("TRN2", target_bir_lowering=False, debug=False)

    q_in = nc.dram_tensor("q_in", [T, C, N], BF, kind="ExternalInput")
    k_in = nc.dram_tensor("k_in", [T, C, N], BF, kind="ExternalInput")
    v_in = nc.dram_tensor("v_in", [T, C, N], BF, kind="ExternalInput")
    w_names = ["wq", "wka", "wkb", "wvt", "woa", "wob"]
    w_dram = {n: nc.dram_tensor(n, [C, C], BF, kind="ExternalInput") for n in w_names}
    sel_dram = nc.dram_tensor("selg", [32, DG * C], BF, kind="ExternalInput")
    g_dram = nc.dram_tensor("gg", [C, DG * 32], BF, kind="ExternalInput")
    cb_dram = nc.dram_tensor("cb", [C, 1], DT, kind="ExternalInput")
    out_dram = nc.dram_tensor("out", [T, C, N], DT, kind="ExternalOutput")
    DBG = _cache.get("debug", False)
    if DBG:
        dbg = {
            "dbg_pt": nc.dram_tensor("dbg_pt", [C, 1020], BF, kind="ExternalOutput"),
            "dbg_scat": nc.dram_tensor("dbg_scat", [C, 2 * N], BF, kind="ExternalOutput"),
            "dbg_den": nc.dram_tensor("dbg_den", [32, 2 * N], DT, kind="ExternalOutput"),
            "dbg_rec": nc.dram_tensor("dbg_rec", [32, 2 * N], DT, kind="ExternalOutput"),
            "dbg_rx": nc.dram_tensor("dbg_rx", [C, 2 * N], DT, kind="ExternalOutput"),
            "dbg_nrm": nc.dram_tensor("dbg_nrm", [C, 2 * N], BF, kind="ExternalOutput"),
        }

    with tile.TileContext(nc) as tc:
        with (
            tc.tile_pool(name="wp", bufs=1) as wp,
            tc.tile_pool(name="io", bufs=3) as io,
            tc.tile_pool(name="qk", bufs=2) as qk,
            tc.tile_pool(name="ptp", bufs=3) as ptp,
            tc.tile_pool(name="scp", bufs=10) as scp,
            tc.tile_pool(name="nrm", bufs=2) as nrm,
            tc.tile_pool(name="dnp", bufs=2) as dnp,
            tc.tile_pool(name="osb", bufs=2) as osb,
            tc.tile_pool(name="eps", bufs=1, space="PSUM") as eps,
            tc.tile_pool(name="scr", bufs=2, space="PSUM") as scr,
            tc.tile_pool(name="avp", bufs=1, space="PSUM") as avp,
            tc.tile_pool(name="dnb", bufs=1, space="PSUM") as dnb,
        ):
            w = {n: wp.tile([C, C], BF, tag=n, name=f"w_{n}") for n in w_names}
            selg = wp.tile([32, DG * C], BF, tag="selg")
            gg = wp.tile([C, DG * 32], BF, tag="gg")
            cb = wp.tile([C, 1], DT, tag="cb")
            for n in w_names:
                nc.sync.dma_start(w[n][:], w_dram[n][:])
            nc.sync.dma_start(selg[:], sel_dram[:])
            nc.sync.dma_start(gg[:], g_dram[:])
            nc.sync.dma_start(cb[:], cb_dram[:])

            # persistent v_aug tiles (double-buffered by t parity); ones
            # columns (17h+16) are set once and never rewritten.
            va0s = [wp.tile([C, 17 * H + 16], BF, tag=f"va0{p}",
                            name=f"va0{p}") for p in range(2)]
            va1s = [wp.tile([C, 17 * H + 16], BF, tag=f"va1{p}",
                            name=f"va1{p}") for p in range(2)]
            for p in range(2):
                nc.vector.memset(va0s[p][:], 0.0)
                nc.vector.memset(va1s[p][:], 0.0)
                v0h = va0s[p][:, 0:17 * H].rearrange("p (h c) -> p h c", h=H)
                v1h = va1s[p][:, 0:17 * H].rearrange("p (h c) -> p h c", h=H)
                nc.vector.memset(v0h[:, :, D:17], 1.0)
                nc.vector.memset(v1h[:, :, D:17], 1.0)

            # per-half energy PSUM free offsets (within a [*,1024) half):
            #   c0 of slot s: s<3 -> 170*s (bank0), s=3 -> 512 (bank1)
            #   c1 pair p=s//2 at 682+170*p, partition base 64*(s%2)
            def c0_off(s):
                return 170 * s if s < 3 else 512

            def c1_off(s):
                return 682 + 170 * (s // 2)

            # pt (SBUF, bf16, [C,1020]) mirrors e with 510 per bank-half:
            def pt_c0(s):
                return 170 * s if s < 3 else 510

            def pt_c1(s):
                return 680 + 170 * (s // 2)

            state = {}

            def emit_proj(tcJ):
                # input DMA + q/k projections for chunk tcJ
                slj = slice(TC * tcJ, TC * (tcJ + 1))
                qTb = io.tile([C, TC * N], BF, tag="qT", name=f"qT{tcJ}")
                kTb = io.tile([C, TC * N], BF, tag="kT", name=f"kT{tcJ}")
                vTb = io.tile([C, TC * N], BF, tag="vT", name=f"vT{tcJ}")
                nc.sync.dma_start(qTb[:].rearrange("c (t n) -> c t n", t=TC),
                                  q_in[slj].rearrange("t c n -> c t n"))
                nc.sync.dma_start(kTb[:].rearrange("c (t n) -> c t n", t=TC),
                                  k_in[slj].rearrange("t c n -> c t n"))
                nc.sync.dma_start(vTb[:].rearrange("c (t n) -> c t n", t=TC),
                                  v_in[slj].rearrange("t c n -> c t n"))
                qsb = qk.tile([C, TC * N], BF, tag="qsb", name=f"qsb{tcJ}")
                kasb = qk.tile([C, TC * N + 24], BF, tag="kasb",
                               name=f"kasb{tcJ}")
                kbsb = qk.tile([C, TC * N + 24], BF, tag="kbsb",
                               name=f"kbsb{tcJ}")
                nc.vector.memset(kasb[:, TC * N:], 0.0)
                nc.vector.memset(kbsb[:, TC * N:], 0.0)
                for wn, src, dst in (("wq", qTb, qsb), ("wka", kTb, kasb),
                                     ("wkb", kTb, kbsb)):
                    pr = scr.tile([C, 512], DT, tag="scr", name=f"pr{tcJ}{wn}")
                    nc.tensor.matmul(pr[:, 0:TC * N], w[wn][:], src[:])
                    nc.vector.tensor_copy(dst[:, 0:TC * N], pr[:, 0:TC * N])
                state[tcJ] = (qsb, kasb, kbsb, vTb)

            for t in range(T):
                tcI, ti = divmod(t, TC)
                gI, gi = divmod(t, DG)
                sl = slice(TC * tcI, TC * (tcI + 1))

                if t == 0:
                    emit_proj(0)
                if ti == 0 and tcI + 1 < T // TC:
                    # prefetch next chunk's inputs + q/k projections so DVE
                    # evacuation bursts never stall the energy matmuls
                    emit_proj(tcI + 1)

                qsb, kasb, kbsb, vTb = state[tcI]
                tn = slice(ti * N, (ti + 1) * N)
                tn0 = slice(ti * N, ti * N + NC0)
                tn1 = slice(ti * N + NC0, (ti + 1) * N)

                # ---- v projection -> node-major v_aug -------------------
                pv = scr.tile([C, 512], DT, tag="scr")
                nc.tensor.matmul(pv[0:NC0, 0:C], vTb[:, tn0], w["wvt"][:])
                nc.tensor.matmul(pv[0:NC1, C:2 * C], vTb[:, tn1], w["wvt"][:],
                                 tile_position=(0, 0))
                nc.tensor.matmul(pv[64:64 + NC1, 2 * C:3 * C], vTb[:, tn1],
                                 w["wvt"][:], tile_position=(0, 64))
                va0, va1 = va0s[t % 2], va1s[t % 2]
                va0_h = va0[:, 0:17 * H].rearrange("p (h c) -> p h c", h=H)
                va1_h = va1[:, 0:17 * H].rearrange("p (h c) -> p h c", h=H)
                nc.vector.tensor_copy(
                    va0_h[:, :, 0:D], pv[:, 0:C].rearrange("p (h c) -> p h c", h=H))
                nc.vector.tensor_copy(
                    va1_h[0:NC1, :, 0:D],
                    pv[0:NC1, C:2 * C].rearrange("p (h c) -> p h c", h=H))
                nc.vector.tensor_copy(
                    va1_h[64:64 + NC1, :, 0:D],
                    pv[64:64 + NC1, 2 * C:3 * C].rearrange("p (h c) -> p h c", h=H))

                # ---- energy + exp, two 4-head halves --------------------
                e = eps.tile([C, 4 * 512], DT, tag="e")
                pts = []
                for u in range(2):
                    ksb = kasb if u == 0 else kbsb
                    base = 1024 * u
                    for s in range(4):
                        rs = slice(32 * s, 32 * s + 32)
                        o0 = base + c0_off(s)
                        nc.tensor.matmul(e[:, o0:o0 + N], ksb[rs, tn0],
                                         qsb[rs, tn], tile_position=(32 * s, 0))
                        pp = 64 * (s % 2)
                        o1 = base + c1_off(s)
                        nc.tensor.matmul(e[pp:pp + 64, o1:o1 + N],
                                         ksb[rs, ti * N + NC0:ti * N + NC0 + 64],
                                         qsb[rs, tn],
                                         tile_position=(32 * s, pp))
                    pt = ptp.tile([C, 1020], BF, tag="pt")
                    for bk in range(2):
                        e_view = e[:, base + 512 * bk:base + 512 * bk + 510
                                   ].rearrange("p (s c) -> p s c", s=3)
                        pt_view = pt[:, 510 * bk:510 * bk + 510].rearrange(
                            "p (s c) -> p s c", s=3)
                        nc.scalar.activation(pt_view, e_view, EXP)
                    pts.append(pt)
                    if DBG and t == 0 and u == 0:
                        nc.sync.dma_start(dbg["dbg_pt"][:], pt[:])

                # ---- attn @ v_aug (col-tiled, ones col -> denominator) --
                av = avp.tile([C, 512], DT, tag="av")
                for u in range(2):
                    pt = pts[u]
                    fo = N * u
                    for s in range(4):
                        h = 4 * u + s
                        co = 32 * s
                        o0 = pt_c0(s)
                        nc.tensor.matmul(
                            av[co:co + 32, fo:fo + N],
                            va0[:, 17 * h:17 * h + 32], pt[:, o0:o0 + N],
                            start=True, stop=False, tile_position=(0, co),
                            skip_group_check=True)
                    for s in range(4):
                        h = 4 * u + s
                        co = 32 * s
                        pp = 64 * (s % 2)
                        o1 = pt_c1(s)
                        nc.tensor.matmul(
                            av[co:co + 32, fo:fo + N],
                            va1[pp:pp + NC1, 17 * h:17 * h + 32],
                            pt[pp:pp + NC1, o1:o1 + N],
                            start=False, stop=True, tile_position=(pp, co),
                            skip_group_check=True)

                # ---- evacuate av, accumulate denominators ---------------
                scat = scp.tile([C, 2 * N], BF, tag="scat")
                nc.scalar.copy(scat[:], av[:, 0:2 * N])
                if gi == 0:
                    den_ps = dnb.tile([32, 2 * N], DT, tag="den", name=f"den{gI}")
                    state[("den", gI)] = den_ps
                den_ps = state[("den", gI)]
                nc.tensor.matmul(den_ps[:], gg[:, 32 * gi:32 * gi + 32], scat[:],
                                 start=(gi == 0), stop=(gi == DG - 1),
                                 skip_group_check=True)
                state[("scat", t)] = scat
                if DBG and t == 0:
                    nc.sync.dma_start(dbg["dbg_scat"][:], scat[:])

                # ---- group flush: reciprocal + back half ----------------
                if gi == DG - 1:
                    den_sb = dnp.tile([32, 2 * N], DT, tag="den_sb")
                    nc.vector.tensor_copy(den_sb[:], den_ps[:])
                    rec = dnp.tile([32, 2 * N], DT, tag="rec")
                    nc.vector.reciprocal_approx_fast(rec[:], den_sb[:])
                    recb = dnp.tile([32, 2 * N], BF, tag="recb")
                    nc.vector.tensor_copy(recb[:], rec[:])
                    if DBG and gI == 0:
                        nc.sync.dma_start(dbg["dbg_den"][:], den_sb[:])
                        nc.sync.dma_start(dbg["dbg_rec"][:], rec[:])

                    for tt in range(gI * DG, (gI + 1) * DG):
                        tcJ, tj = divmod(tt, TC)
                        ii = tt % DG
                        rx = scr.tile([C, 512], DT, tag="scr")
                        nc.tensor.matmul(rx[:, 0:2 * N],
                                         selg[:, C * ii:C * ii + C], recb[:])
                        if tj == 0:
                            state[("nrm", tcJ)] = nrm.tile([C, TC * 2 * N], BF,
                                                           tag="normt",
                                                           name=f"nrm{tcJ}")
                        normt = state[("nrm", tcJ)]
                        nc.vector.tensor_tensor(
                            normt[:, tj * 2 * N:(tj + 1) * 2 * N],
                            state[("scat", tt)][:], rx[:, 0:2 * N],
                            mybir.AluOpType.mult)
                        if DBG and tt == 0:
                            rxdbg = nrm.tile([C, 2 * N], DT, tag="rxdbg")
                            nc.vector.tensor_copy(rxdbg[:], rx[:, 0:2 * N])
                            nc.sync.dma_start(dbg["dbg_rx"][:], rxdbg[:])
                            nc.sync.dma_start(dbg["dbg_nrm"][:],
                                              normt[:, 0:2 * N])

                        if tj == TC - 1:
                            fin = scr.tile([C, 512], DT, tag="scr")
                            na = normt[:].rearrange(
                                "p (t g n) -> p t g n", t=TC, g=2)
                            nc.tensor.matmul(fin[:, 0:TC * N], w["woa"][:],
                                             na[:, :, 0, :], start=True,
                                             stop=False)
                            nc.tensor.matmul(fin[:, 0:TC * N], w["wob"][:],
                                             na[:, :, 1, :], start=False,
                                             stop=True)
                            out_sb = osb.tile([C, TC * N], DT, tag="out_sb")
                            nc.vector.tensor_scalar_add(out_sb[:],
                                                        fin[:, 0:TC * N],
                                                        cb[:, 0:1])
                            osl = slice(TC * tcJ, TC * (tcJ + 1))
                            nc.sync.dma_start(
                                out_dram[osl].rearrange("t c n -> c t n"),
                                out_sb[:].rearrange("c (t n) -> c t n", t=TC))

    nc.compile()
    return nc


def _prep_weights(Wv, bv, Wk, bk, Wq, bq, Wo, bo):
    s = F32(1.0 / np.sqrt(C))
    Wq_s = (Wq * s).astype(F32)

    if np.any(bq) or np.any(bk):
        raise NotImplementedError("nonzero q/k biases not folded")

    # head h -> rows 32*(h%4) + 16*(h//4) + d
    wq = np.zeros((C, C), F32)
    wka = np.zeros((C, C), F32)
    wkb = np.zeros((C, C), F32)
    for h in range(H):
        for d in range(D):
            r = 32 * (h % 4) + 16 * (h // 4) + d
            wq[:, r] = Wq_s[D * h + d, :]
            (wka if h < 4 else wkb)[:, r] = Wk[D * h + d, :]

    # output projection: norm rows 32j+r (r<16) hold head j (woa) / head
    # 4+j (wob); rows 32j+16..31 stay zero.
    woa = np.zeros((C, C), F32)
    wob = np.zeros((C, C), F32)
    for j in range(4):
        for r in range(D):
            woa[32 * j + r, :] = Wo[:, D * j + r]
            wob[32 * j + r, :] = Wo[:, D * (4 + j) + r]

    # selector/gather matrices per in-group index i:
    #   gather G_i[C, 32]: den row 4i+j <- scat partition 32j+16
    #   select S_i[32, C]: rx rows 32j..32j+16 <- rec row 4i+j
    selg = np.zeros((32, DG * C), F32)
    gg = np.zeros((C, DG * 32), F32)
    for i in range(DG):
        for j in range(4):
            selg[4 * i + j, C * i + 32 * j:C * i + 32 * j + 17] = 1.0
            gg[32 * j + 16, 32 * i + 4 * i + j] = 1.0

    cb = (bo + Wo @ bv).astype(F32).reshape(C, 1)

    np_bf = mybir.dt.np(BF)
    wm = dict(wq=wq, wka=wka, wkb=wkb,
              wvt=np.ascontiguousarray(Wv.T).astype(F32), woa=woa, wob=wob)
    wm = {k: v.astype(np_bf) for k, v in wm.items()}
    wm["selg"] = selg.astype(np_bf)
    wm["gg"] = gg.astype(np_bf)
    wm["cb"] = cb
    return wm


def kernel(values, keys, query, Wv, bv, Wk, bk, Wq, bq, Wo, bo):
    values = np.asarray(values, F32)
    keys = np.asarray(keys, F32)
    query = np.asarray(query, F32)

    if "nc" not in _cache:
        _cache["nc"] = _build_kernel()
    nc = _cache["nc"]

    wmap = _prep_weights(np.asarray(Wv, F32), np.asarray(bv, F32),
                         np.asarray(Wk, F32), np.asarray(bk, F32),
                         np.asarray(Wq, F32), np.asarray(bq, F32),
                         np.asarray(Wo, F32), np.asarray(bo, F32))

    np_bf = mybir.dt.np(BF)
    qT = np.ascontiguousarray(query.transpose(0, 1, 3, 2)).astype(np_bf)
    kT = np.ascontiguousarray(keys.transpose(0, 1, 3, 2)).astype(np_bf)
    vT = np.ascontiguousarray(values.transpose(0, 1, 3, 2)).astype(np_bf)

    in_maps = [
        dict(q_in=qT[b], k_in=kT[b], v_in=vT[b], **wmap) for b in range(B)
    ]
    results = run_bass_kernel_spmd(nc, in_maps, list(range(B))).results
    out = np.stack([r["out"] for r in results])             # [B, T, C, N]
    return np.ascontiguousarray(out.transpose(0, 1, 3, 2))  # [B, T, N, C]


# revision 19
# speedup vs baseline: 1.0349x; 1.0349x over previous
"""Spatial multi-head attention kernel for Trainium2 (8 NeuronCores).

Problem: B=8, T=64, N=170 nodes, C=128 channels, H=8 heads, D=16.
Attention over nodes N, independent per (b, t, h).

Strategy (v2):
- Pure data-parallel over B: core b computes batch b fully (no collectives).
- Host transposes inputs to channel-major [B, T, C, N] bf16; output returned
  channel-major [T, C, N] f32 per core and transposed back on host.
- Interleaved head layout: head h lives at partition rows
  32*(h%4) + 16*(h//4) + d.  One q projection (wq holds all 8 heads);
  k is projected twice (wka: heads 0-3 with heads 4-7 columns zeroed, wkb:
  the complement) so the K=32 energy matmuls contract only the right head.
- Energy per timestep packed dense into 4 PSUM banks as two 4-head halves;
  each half exp'd in a single ACT instruction ([128, 2, 3, 170] view).
  Only the Exp table set is ever loaded (reciprocal is done on DVE).
- attn@v via col-tiled M=17 matmuls with a ones column appended to v
  (denominator lands in partition 32j+16 of the av bank).
- Denominators for 8 timesteps gathered by matmul (G_i, accumulating) into
  one PSUM bank, reciprocal'd once per group with DVE
  reciprocal_approx_fast, then expanded per timestep with a selector
  matmul; normalization is one DVE multiply per timestep.
- Projections / output projection / DMA batched over 2-timestep chunks.
"""
import sys

sys.path.insert(0, "/opt/trn_rl_repo")

import numpy as np

import concourse.bacc as bacc
import concourse.mybir as mybir
import concourse.tile as tile
from concourse.bass_utils import run_bass_kernel_spmd

B, T, N, C = 8, 64, 170, 128
H, D = 8, 16
NC0 = 128            # first key chunk
NC1 = N - NC0        # 42
TC = 2               # timesteps per proj/io chunk
DG = 8               # timesteps per denominator group
F32 = np.float32
DT = mybir.dt.float32
BF = mybir.dt.bfloat16
EXP = mybir.ActivationFunctionType.Exp

_cache = {}


def _build_kernel():
    nc = bacc.Bacc("TRN2", target_bir_lowering=False, debug=False)

    q_in = nc.dram_tensor("q_in", [T, C, N], BF, kind="ExternalInput")
    k_in = nc.dram_tensor("k_in", [T, C, N], BF, kind="ExternalInput")
    v_in = nc.dram_tensor("v_in", [T, C, N], BF, kind="ExternalInput")
    w_names = ["wq", "wka", "wkb", "wvt", "woa", "wob"]
    w_dram = {n: nc.dram_tensor(n, [C, C], BF, kind="ExternalInput") for n in w_names}
    sel_dram = nc.dram_tensor("selg", [32, DG * C], BF, kind="ExternalInput")
    g_dram = nc.dram_tensor("gg", [C, DG * 32], BF, kind="ExternalInput")
    cb_dram = nc.dram_tensor("cb", [C, 1], DT, kind="ExternalInput")
    out_dram = nc.dram_tensor("out", [T, C, N], DT, kind="ExternalOutput")
    DBG = _cache.get("debug", False)
    if DBG:
        dbg = {
            "dbg_pt": nc.dram_tensor("dbg_pt", [C, 1020], BF, kind="ExternalOutput"),
            "dbg_scat": nc.dram_tensor("dbg_scat", [C, 2 * N], BF, kind="ExternalOutput"),
            "dbg_den": nc.dram_tensor("dbg_den", [32, 2 * N], DT, kind="ExternalOutput"),
            "dbg_rec": nc.dram_tensor("dbg_rec", [32, 2 * N], DT, kind="ExternalOutput"),
            "dbg_rx": nc.dram_tensor("dbg_rx", [C, 2 * N], DT, kind="ExternalOutput"),
            "dbg_nrm": nc.dram_tensor("dbg_nrm", [C, 2 * N], BF, kind="ExternalOutput"),
        }

    with tile.TileContext(nc) as tc:
        with (
            tc.tile_pool(name="wp", bufs=1) as wp,
            tc.tile_pool(name="io", bufs=3) as io,
            tc.tile_pool(name="qk", bufs=2) as qk,
            tc.tile_pool(name="ptp", bufs=3) as ptp,
            tc.tile_pool(name="scp", bufs=10) as scp,
            tc.tile_pool(name="nrm", bufs=2) as nrm,
            tc.tile_pool(name="dnp", bufs=2) as dnp,
            tc.tile_pool(name="osb", bufs=2) as osb,
            tc.tile_pool(name="eps", bufs=1, space="PSUM") as eps,
            tc.tile_pool(name="scr", bufs=2, space="PSUM") as scr,
            tc.tile_pool(name="avp", bufs=1, space="PSUM") as avp,
            tc.tile_pool(name="dnb", bufs=1, space="PSUM") as dnb,
        ):
            w = {n: wp.tile([C, C], BF, tag=n, name=f"w_{n}") for n in w_names}
            selg = wp.tile([32, DG * C], BF, tag="selg")
            gg = wp.tile([C, DG * 32], BF, tag="gg")
            cb = wp.tile([C, 1], DT, tag="cb")
            for n in w_names:
                nc.sync.dma_start(w[n][:], w_dram[n][:])
            nc.sync.dma_start(selg[:], sel_dram[:])
            nc.sync.dma_start(gg[:], g_dram[:])
            nc.sync.dma_start(cb[:], cb_dram[:])

            # persistent v_aug tiles (double-buffered by t parity); ones
            # columns (17h+16) are set once and never rewritten.
            va0s = [wp.tile([C, 17 * H + 16], BF, tag=f"va0{p}",
                            name=f"va0{p}") for p in range(2)]
            va1s = [wp.tile([C, 17 * H + 16], BF, tag=f"va1{p}",
                            name=f"va1{p}") for p in range(2)]
            for p in range(2):
                nc.vector.memset(va0s[p][:], 0.0)
                nc.vector.memset(va1s[p][:], 0.0)
                v0h = va0s[p][:, 0:17 * H].rearrange("p (h c) -> p h c", h=H)
                v1h = va1s[p][:, 0:17 * H].rearrange("p (h c) -> p h c", h=H)
                nc.vector.memset(v0h[:, :, D:17], 1.0)
                nc.vector.memset(v1h[:, :, D:17], 1.0)

            # per-half energy PSUM free offsets (within a [*,1024) half):
            #   c0 of slot s: s<3 -> 170*s (bank0), s=3 -> 512 (bank1)
            #   c1 pair p=s//2 at 682+170*p, partition base 64*(s%2)
            def c0_off(s):
                return 170 * s if s < 3 else 512

            def c1_off(s):
                return 682 + 170 * (s // 2)

            # pt (SBUF, bf16, [C,1020]) mirrors e with 510 per bank-half:
            def pt_c0(s):
                return 170 * s if s < 3 else 510

            def pt_c1(s):
                return 680 + 170 * (s // 2)

            state = {}

            def emit_proj(tcJ):
                # input DMA + q/k projections for chunk tcJ
                slj = slice(TC * tcJ, TC * (tcJ + 1))
                qTb = io.tile([C, TC * N], BF, tag="qT", name=f"qT{tcJ}")
                kTb = io.tile([C, TC * N], BF, tag="kT", name=f"kT{tcJ}")
                vTb = io.tile([C, TC * N], BF, tag="vT", name=f"vT{tcJ}")
                nc.sync.dma_start(qTb[:].rearrange("c (t n) -> c t n", t=TC),
                                  q_in[slj].rearrange("t c n -> c t n"))
                nc.sync.dma_start(kTb[:].rearrange("c (t n) -> c t n", t=TC),
                                  k_in[slj].rearrange("t c n -> c t n"))
                nc.sync.dma_start(vTb[:].rearrange("c (t n) -> c t n", t=TC),
                                  v_in[slj].rearrange("t c n -> c t n"))
                qsb = qk.tile([C, TC * N], BF, tag="qsb", name=f"qsb{tcJ}")
                kasb = qk.tile([C, TC * N + 24], BF, tag="kasb",
                               name=f"kasb{tcJ}")
                kbsb = qk.tile([C, TC * N + 24], BF, tag="kbsb",
                               name=f"kbsb{tcJ}")
                nc.vector.memset(kasb[:, TC * N:], 0.0)
                nc.vector.memset(kbsb[:, TC * N:], 0.0)
                for wn, src, dst in (("wq", qTb, qsb), ("wka", kTb, kasb),
                                     ("wkb", kTb, kbsb)):
                    pr = scr.tile([C, 512], DT, tag="scr", name=f"pr{tcJ}{wn}")
                    nc.tensor.matmul(pr[:, 0:TC * N], w[wn][:], src[:])
                    nc.vector.tensor_copy(dst[:, 0:TC * N], pr[:, 0:TC * N])
                state[tcJ] = (qsb, kasb, kbsb, vTb)

            for t in range(T):
                tcI, ti = divmod(t, TC)
                gI, gi = divmod(t, DG)
                sl = slice(TC * tcI, TC * (tcI + 1))

                if t == 0:
                    emit_proj(0)
                if ti == 0 and tcI + 1 < T // TC:
                    # prefetch next chunk's inputs + q/k projections so DVE
                    # evacuation bursts never stall the energy matmuls
                    emit_proj(tcI + 1)

                qsb, kasb, kbsb, vTb = state[tcI]
                tn = slice(ti * N, (ti + 1) * N)
                tn0 = slice(ti * N, ti * N + NC0)
                tn1 = slice(ti * N + NC0, (ti + 1) * N)

                # ---- v projection -> node-major v_aug -------------------
                pv = scr.tile([C, 512], DT, tag="scr")
                nc.tensor.matmul(pv[0:NC0, 0:C], vTb[:, tn0], w["wvt"][:])
                nc.tensor.matmul(pv[0:NC1, C:2 * C], vTb[:, tn1], w["wvt"][:],
                                 tile_position=(0, 0))
                nc.tensor.matmul(pv[64:64 + NC1, 2 * C:3 * C], vTb[:, tn1],
                                 w["wvt"][:], tile_position=(0, 64))
                va0, va1 = va0s[t % 2], va1s[t % 2]
                va0_h = va0[:, 0:17 * H].rearrange("p (h c) -> p h c", h=H)
                va1_h = va1[:, 0:17 * H].rearrange("p (h c) -> p h c", h=H)
                nc.vector.tensor_copy(
                    va0_h[:, :, 0:D], pv[:, 0:C].rearrange("p (h c) -> p h c", h=H))
                nc.vector.tensor_copy(
                    va1_h[0:NC1, :, 0:D],
                    pv[0:NC1, C:2 * C].rearrange("p (h c) -> p h c", h=H))
                nc.vector.tensor_copy(
                    va1_h[64:64 + NC1, :, 0:D],
                    pv[64:64 + NC1, 2 * C:3 * C].rearrange("p (h c) -> p h c", h=H))

                # ---- energy + exp, two 4-head halves --------------------
                e = eps.tile([C, 4 * 512], DT, tag="e")
                pts = []
                for u in range(2):
                    ksb = kasb if u == 0 else kbsb
                    base = 1024 * u
                    for s in range(4):
                        rs = slice(32 * s, 32 * s + 32)
                        o0 = base + c0_off(s)
                        nc.tensor.matmul(e[:, o0:o0 + N], ksb[rs, tn0],
                                         qsb[rs, tn], tile_position=(32 * s, 0))
                        pp = 64 * (s % 2)
                        o1 = base + c1_off(s)
                        nc.tensor.matmul(e[pp:pp + 64, o1:o1 + N],
                                         ksb[rs, ti * N + NC0:ti * N + NC0 + 64],
                                         qsb[rs, tn],
                                         tile_position=(32 * s, pp))
                    pt = ptp.tile([C, 1020], BF, tag="pt")
                    e_view = e[:, base:base + 1024].rearrange(
                        "p (b c) -> p b c", b=2)[:, :, 0:510].rearrange(
                        "p b (s c) -> p b s c", s=3)
                    pt_view = pt[:].rearrange("p (b c) -> p b c", b=2).rearrange(
                        "p b (s c) -> p b s c", s=3)
                    nc.scalar.activation(pt_view, e_view, EXP)
                    pts.append(pt)
                    if DBG and t == 0 and u == 0:
                        nc.sync.dma_start(dbg["dbg_pt"][:], pt[:])

                # ---- attn @ v_aug (col-tiled, ones col -> denominator) --
                av = avp.tile([C, 512], DT, tag="av")
                for u in range(2):
                    pt = pts[u]
                    fo = N * u
                    for s in range(4):
                        h = 4 * u + s
                        co = 32 * s
                        o0 = pt_c0(s)
                        nc.tensor.matmul(
                            av[co:co + 32, fo:fo + N],
                            va0[:, 17 * h:17 * h + 32], pt[:, o0:o0 + N],
                            start=True, stop=False, tile_position=(0, co),
                            skip_group_check=True)
                    for s in range(4):
                        h = 4 * u + s
                        co = 32 * s
                        pp = 64 * (s % 2)
                        o1 = pt_c1(s)
                        nc.tensor.matmul(
                            av[co:co + 32, fo:fo + N],
                            va1[pp:pp + NC1, 17 * h:17 * h + 32],
                            pt[pp:pp + NC1, o1:o1 + N],
                            start=False, stop=True, tile_position=(pp, co),
                            skip_group_check=True)

                # ---- evacuate av, accumulate denominators ---------------
                scat = scp.tile([C, 2 * N], BF, tag="scat")
                nc.scalar.copy(scat[:], av[:, 0:2 * N])
                if gi == 0:
                    den_ps = dnb.tile([32, 2 * N], DT, tag="den", name=f"den{gI}")
                    state[("den", gI)] = den_ps
                den_ps = state[("den", gI)]
                nc.tensor.matmul(den_ps[:], gg[:, 32 * gi:32 * gi + 32], scat[:],
                                 start=(gi == 0), stop=(gi == DG - 1),
                                 skip_group_check=True)
                state[("scat", t)] = scat
                if DBG and t == 0:
                    nc.sync.dma_start(dbg["dbg_scat"][:], scat[:])

                # ---- group flush: reciprocal + back half ----------------
                if gi == DG - 1:
                    den_sb = dnp.tile([32, 2 * N], DT, tag="den_sb")
                    nc.vector.tensor_copy(den_sb[:], den_ps[:])
                    rec = dnp.tile([32, 2 * N], DT, tag="rec")
                    nc.vector.reciprocal_approx_fast(rec[:], den_sb[:])
                    recb = dnp.tile([32, 2 * N], BF, tag="recb")
                    nc.vector.tensor_copy(recb[:], rec[:])
                    if DBG and gI == 0:
                        nc.sync.dma_start(dbg["dbg_den"][:], den_sb[:])
                        nc.sync.dma_start(dbg["dbg_rec"][:], rec[:])

                    for tt in range(gI * DG, (gI + 1) * DG):
                        tcJ, tj = divmod(tt, TC)
                        ii = tt % DG
                        rx = scr.tile([C, 512], DT, tag="scr")
                        nc.tensor.matmul(rx[:, 0:2 * N],
                                         selg[:, C * ii:C * ii + C], recb[:])
                        if tj == 0:
                            state[("nrm", tcJ)] = nrm.tile([C, TC * 2 * N], BF,
                                                           tag="normt",
                                                           name=f"nrm{tcJ}")
                        normt = state[("nrm", tcJ)]
                        nc.vector.tensor_tensor(
                            normt[:, tj * 2 * N:(tj + 1) * 2 * N],
                            state[("scat", tt)][:], rx[:, 0:2 * N],
                            mybir.AluOpType.mult)
                        if DBG and tt == 0:
                            rxdbg = nrm.tile([C, 2 * N], DT, tag="rxdbg")
                            nc.vector.tensor_copy(rxdbg[:], rx[:, 0:2 * N])
                            nc.sync.dma_start(dbg["dbg_rx"][:], rxdbg[:])
                            nc.sync.dma_start(dbg["dbg_nrm"][:],
                                              normt[:, 0:2 * N])

                        if tj == TC - 1:
                            fin = scr.tile([C, 512], DT, tag="scr")
                            na = normt[:].rearrange(
                                "p (t g n) -> p t g n", t=TC, g=2)
                            nc.tensor.matmul(fin[:, 0:TC * N], w["woa"][:],
                                             na[:, :, 0, :], start=True,
                                             stop=False)
                            nc.tensor.matmul(fin[:, 0:TC * N], w["wob"][:],
                                             na[:, :, 1, :], start=False,
                                             stop=True)
                            out_sb = osb.tile([C, TC * N], DT, tag="out_sb")
                            nc.vector.tensor_scalar_add(out_sb[:],
                                                        fin[:, 0:TC * N],
                                                        cb[:, 0:1])
                            osl = slice(TC * tcJ, TC * (tcJ + 1))
                            nc.sync.dma_start(
                                out_dram[osl].rearrange("t c n -> c t n"),
                                out_sb[:].rearrange("c (t n) -> c t n", t=TC))

    nc.compile()
    return nc


def _prep_weights(Wv, bv, Wk, bk, Wq, bq, Wo, bo):
    s = F32(1.0 / np.sqrt(C))
    Wq_s = (Wq * s).astype(F32)

    if np.any(bq) or np.any(bk):
        raise NotImplementedError("nonzero q/k biases not folded")

    # head h -> rows 32*(h%4) + 16*(h//4) + d
    wq = np.zeros((C, C), F32)
    wka = np.zeros((C, C), F32)
    wkb = np.zeros((C, C), F32)
    for h in range(H):
        for d in range(D):
            r = 32 * (h % 4) + 16 * (h // 4) + d
            wq[:, r] = Wq_s[D * h + d, :]
            (wka if h < 4 else wkb)[:, r] = Wk[D * h + d, :]

    # output projection: norm rows 32j+r (r<16) hold head j (woa) / head
    # 4+j (wob); rows 32j+16..31 stay zero.
    woa = np.zeros((C, C), F32)
    wob = np.zeros((C, C), F32)
    for j in range(4):
        for r in range(D):
            woa[32 * j + r, :] = Wo[:, D * j + r]
            wob[32 * j + r, :] = Wo[:, D * (4 + j) + r]

    # selector/gather matrices per in-group index i:
    #   gather G_i[C, 32]: den row 4i+j <- scat partition 32j+16
    #   select S_i[32, C]: rx rows 32j..32j+16 <- rec row 4i+j
    selg = np.zeros((32, DG * C), F32)
    gg = np.zeros((C, DG * 32), F32)
    for i in range(DG):
        for j in range(4):
            selg[4 * i + j, C * i + 32 * j:C * i + 32 * j + 17] = 1.0
            gg[32 * j + 16, 32 * i + 4 * i + j] = 1.0

    cb = (bo + Wo @ bv).astype(F32).reshape(C, 1)

    np_bf = mybir.dt.np(BF)
    wm = dict(wq=wq, wka=wka, wkb=wkb,
              wvt=np.ascontiguousarray(Wv.T).astype(F32), woa=woa, wob=wob)
    wm = {k: v.astype(np_bf) for k, v in wm.items()}
    wm["selg"] = selg.astype(np_bf)
    wm["gg"] = gg.astype(np_bf)
    wm["cb"] = cb
    return wm


def kernel(values, keys, query, Wv, bv, Wk, bk, Wq, bq, Wo, bo):
    values = np.asarray(values, F32)
    keys = np.asarray(keys, F32)
    query = np.asarray(query, F32)

    if "nc" not in _cache:
        _cache["nc"] = _build_kernel()
    nc = _cache["nc"]

    wmap = _prep_weights(np.asarray(Wv, F32), np.asarray(bv, F32),
                         np.asarray(Wk, F32), np.asarray(bk, F32),
                         np.asarray(Wq, F32), np.asarray(bq, F32),
                         np.asarray(Wo, F32), np.asarray(bo, F32))

    np_bf = mybir.dt.np(BF)
    qT = np.ascontiguousarray(query.transpose(0, 1, 3, 2)).astype(np_bf)
    kT = np.ascontiguousarray(keys.transpose(0, 1, 3, 2)).astype(np_bf)
    vT = np.ascontiguousarray(values.transpose(0, 1, 3, 2)).astype(np_bf)

    in_maps = [
        dict(q_in=qT[b], k_in=kT[b], v_in=vT[b], **wmap) for b in range(B)
    ]
    results = run_bass_kernel_spmd(nc, in_maps, list(range(B))).results
    out = np.stack([r["out"] for r in results])             # [B, T, C, N]
    return np.ascontiguousarray(out.transpose(0, 1, 3, 2))  # [B, T, N, C]


# revision 20
# speedup vs baseline: 1.0515x; 1.0160x over previous
"""Spatial multi-head attention kernel for Trainium2 (8 NeuronCores).

Problem: B=8, T=64, N=170 nodes, C=128 channels, H=8 heads, D=16.
Attention over nodes N, independent per (b, t, h).

Strategy (v2):
- Pure data-parallel over B: core b computes batch b fully (no collectives).
- Host transposes inputs to channel-major [B, T, C, N] bf16; output returned
  channel-major [T, C, N] f32 per core and transposed back on host.
- Interleaved head layout: head h lives at partition rows
  32*(h%4) + 16*(h//4) + d.  One q projection (wq holds all 8 heads);
  k is projected twice (wka: heads 0-3 with heads 4-7 columns zeroed, wkb:
  the complement) so the K=32 energy matmuls contract only the right head.
- Energy per timestep packed dense into 4 PSUM banks as two 4-head halves;
  each half exp'd in a single ACT instruction ([128, 2, 3, 170] view).
  Only the Exp table set is ever loaded (reciprocal is done on DVE).
- attn@v via col-tiled M=17 matmuls with a ones column appended to v
  (denominator lands in partition 32j+16 of the av bank).
- Denominators for 8 timesteps gathered by matmul (G_i, accumulating) into
  one PSUM bank, reciprocal'd once per group with DVE
  reciprocal_approx_fast, then expanded per timestep with a selector
  matmul; normalization is one DVE multiply per timestep.
- Projections / output projection / DMA batched over 2-timestep chunks.
"""
import sys

sys.path.insert(0, "/opt/trn_rl_repo")

import numpy as np

import concourse.bacc as bacc
import concourse.mybir as mybir
import concourse.tile as tile
from concourse.bass_utils import run_bass_kernel_spmd

B, T, N, C = 8, 64, 170, 128
H, D = 8, 16
NC0 = 128            # first key chunk
NC1 = N - NC0        # 42
TC = 2               # timesteps per proj/io chunk
DG = 8               # timesteps per denominator group
F32 = np.float32
DT = mybir.dt.float32
BF = mybir.dt.bfloat16
EXP = mybir.ActivationFunctionType.Exp

_cache = {}


def _build_kernel():
    nc = bacc.Bacc("TRN2", target_bir_lowering=False, debug=False)

    q_in = nc.dram_tensor("q_in", [T, C, N], BF, kind="ExternalInput")
    k_in = nc.dram_tensor("k_in", [T, C, N], BF, kind="ExternalInput")
    v_in = nc.dram_tensor("v_in", [T, C, N], BF, kind="ExternalInput")
    w_names = ["wq", "wka", "wkb", "wvt", "woa", "wob"]
    w_dram = {n: nc.dram_tensor(n, [C, C], BF, kind="ExternalInput") for n in w_names}
    sel_dram = nc.dram_tensor("selg", [32, DG * C], BF, kind="ExternalInput")
    g_dram = nc.dram_tensor("gg", [C, DG * 32], BF, kind="ExternalInput")
    cb_dram = nc.dram_tensor("cb", [C, 1], DT, kind="ExternalInput")
    out_dram = nc.dram_tensor("out", [T, C, N], DT, kind="ExternalOutput")
    DBG = _cache.get("debug", False)
    if DBG:
        dbg = {
            "dbg_pt": nc.dram_tensor("dbg_pt", [C, 1020], BF, kind="ExternalOutput"),
            "dbg_scat": nc.dram_tensor("dbg_scat", [C, 2 * N], BF, kind="ExternalOutput"),
            "dbg_den": nc.dram_tensor("dbg_den", [32, 2 * N], DT, kind="ExternalOutput"),
            "dbg_rec": nc.dram_tensor("dbg_rec", [32, 2 * N], DT, kind="ExternalOutput"),
            "dbg_rx": nc.dram_tensor("dbg_rx", [C, 2 * N], DT, kind="ExternalOutput"),
            "dbg_nrm": nc.dram_tensor("dbg_nrm", [C, 2 * N], BF, kind="ExternalOutput"),
        }

    with tile.TileContext(nc) as tc:
        with (
            tc.tile_pool(name="wp", bufs=1) as wp,
            tc.tile_pool(name="io", bufs=3) as io,
            tc.tile_pool(name="qk", bufs=3) as qk,
            tc.tile_pool(name="ptp", bufs=4) as ptp,
            tc.tile_pool(name="scp", bufs=10) as scp,
            tc.tile_pool(name="nrm", bufs=2) as nrm,
            tc.tile_pool(name="dnp", bufs=2) as dnp,
            tc.tile_pool(name="osb", bufs=2) as osb,
            tc.tile_pool(name="eps", bufs=1, space="PSUM") as eps,
            tc.tile_pool(name="scr", bufs=2, space="PSUM") as scr,
            tc.tile_pool(name="avp", bufs=1, space="PSUM") as avp,
            tc.tile_pool(name="dnb", bufs=1, space="PSUM") as dnb,
        ):
            w = {n: wp.tile([C, C], BF, tag=n, name=f"w_{n}") for n in w_names}
            selg = wp.tile([32, DG * C], BF, tag="selg")
            gg = wp.tile([C, DG * 32], BF, tag="gg")
            cb = wp.tile([C, 1], DT, tag="cb")
            for n in w_names:
                nc.sync.dma_start(w[n][:], w_dram[n][:])
            nc.sync.dma_start(selg[:], sel_dram[:])
            nc.sync.dma_start(gg[:], g_dram[:])
            nc.sync.dma_start(cb[:], cb_dram[:])

            # persistent v_aug tiles (double-buffered by t parity); ones
            # columns (17h+16) are set once and never rewritten.
            va0s = [wp.tile([C, 17 * H + 16], BF, tag=f"va0{p}",
                            name=f"va0{p}") for p in range(2)]
            va1s = [wp.tile([C, 17 * H + 16], BF, tag=f"va1{p}",
                            name=f"va1{p}") for p in range(2)]
            for p in range(2):
                nc.vector.memset(va0s[p][:], 0.0)
                nc.vector.memset(va1s[p][:], 0.0)
                v0h = va0s[p][:, 0:17 * H].rearrange("p (h c) -> p h c", h=H)
                v1h = va1s[p][:, 0:17 * H].rearrange("p (h c) -> p h c", h=H)
                nc.vector.memset(v0h[:, :, D:17], 1.0)
                nc.vector.memset(v1h[:, :, D:17], 1.0)

            # per-half energy PSUM free offsets (within a [*,1024) half):
            #   c0 of slot s: s<3 -> 170*s (bank0), s=3 -> 512 (bank1)
            #   c1 pair p=s//2 at 682+170*p, partition base 64*(s%2)
            def c0_off(s):
                return 170 * s if s < 3 else 512

            def c1_off(s):
                return 682 + 170 * (s // 2)

            # pt (SBUF, bf16, [C,1020]) mirrors e with 510 per bank-half:
            def pt_c0(s):
                return 170 * s if s < 3 else 510

            def pt_c1(s):
                return 680 + 170 * (s // 2)

            state = {}

            def emit_proj(tcJ):
                # input DMA + q/k projections for chunk tcJ
                slj = slice(TC * tcJ, TC * (tcJ + 1))
                qTb = io.tile([C, TC * N], BF, tag="qT", name=f"qT{tcJ}")
                kTb = io.tile([C, TC * N], BF, tag="kT", name=f"kT{tcJ}")
                vTb = io.tile([C, TC * N], BF, tag="vT", name=f"vT{tcJ}")
                nc.sync.dma_start(qTb[:].rearrange("c (t n) -> c t n", t=TC),
                                  q_in[slj].rearrange("t c n -> c t n"))
                nc.sync.dma_start(kTb[:].rearrange("c (t n) -> c t n", t=TC),
                                  k_in[slj].rearrange("t c n -> c t n"))
                nc.sync.dma_start(vTb[:].rearrange("c (t n) -> c t n", t=TC),
                                  v_in[slj].rearrange("t c n -> c t n"))
                qsb = qk.tile([C, TC * N], BF, tag="qsb", name=f"qsb{tcJ}")
                kasb = qk.tile([C, TC * N + 24], BF, tag="kasb",
                               name=f"kasb{tcJ}")
                kbsb = qk.tile([C, TC * N + 24], BF, tag="kbsb",
                               name=f"kbsb{tcJ}")
                nc.vector.memset(kasb[:, TC * N:], 0.0)
                nc.vector.memset(kbsb[:, TC * N:], 0.0)
                for wn, src, dst in (("wq", qTb, qsb), ("wka", kTb, kasb),
                                     ("wkb", kTb, kbsb)):
                    pr = scr.tile([C, 512], DT, tag="scr", name=f"pr{tcJ}{wn}")
                    nc.tensor.matmul(pr[:, 0:TC * N], w[wn][:], src[:])
                    nc.vector.tensor_copy(dst[:, 0:TC * N], pr[:, 0:TC * N])
                state[tcJ] = (qsb, kasb, kbsb, vTb)

            for t in range(T):
                tcI, ti = divmod(t, TC)
                gI, gi = divmod(t, DG)
                sl = slice(TC * tcI, TC * (tcI + 1))

                if t == 0:
                    emit_proj(0)
                if ti == 0 and tcI + 1 < T // TC:
                    # prefetch next chunk's inputs + q/k projections so DVE
                    # evacuation bursts never stall the energy matmuls
                    emit_proj(tcI + 1)

                qsb, kasb, kbsb, vTb = state[tcI]
                tn = slice(ti * N, (ti + 1) * N)
                tn0 = slice(ti * N, ti * N + NC0)
                tn1 = slice(ti * N + NC0, (ti + 1) * N)

                # ---- v projection -> node-major v_aug -------------------
                pv = scr.tile([C, 512], DT, tag="scr")
                nc.tensor.matmul(pv[0:NC0, 0:C], vTb[:, tn0], w["wvt"][:])
                nc.tensor.matmul(pv[0:NC1, C:2 * C], vTb[:, tn1], w["wvt"][:],
                                 tile_position=(0, 0))
                nc.tensor.matmul(pv[64:64 + NC1, 2 * C:3 * C], vTb[:, tn1],
                                 w["wvt"][:], tile_position=(0, 64))
                va0, va1 = va0s[t % 2], va1s[t % 2]
                va0_h = va0[:, 0:17 * H].rearrange("p (h c) -> p h c", h=H)
                va1_h = va1[:, 0:17 * H].rearrange("p (h c) -> p h c", h=H)
                nc.vector.tensor_copy(
                    va0_h[:, :, 0:D], pv[:, 0:C].rearrange("p (h c) -> p h c", h=H))
                nc.vector.tensor_copy(
                    va1_h[0:NC1, :, 0:D],
                    pv[0:NC1, C:2 * C].rearrange("p (h c) -> p h c", h=H))
                nc.vector.tensor_copy(
                    va1_h[64:64 + NC1, :, 0:D],
                    pv[64:64 + NC1, 2 * C:3 * C].rearrange("p (h c) -> p h c", h=H))

                # ---- energy + exp, two 4-head halves --------------------
                e = eps.tile([C, 4 * 512], DT, tag="e")
                pts = []
                for u in range(2):
                    ksb = kasb if u == 0 else kbsb
                    base = 1024 * u
                    for s in range(4):
                        rs = slice(32 * s, 32 * s + 32)
                        o0 = base + c0_off(s)
                        nc.tensor.matmul(e[:, o0:o0 + N], ksb[rs, tn0],
                                         qsb[rs, tn], tile_position=(32 * s, 0))
                        pp = 64 * (s % 2)
                        o1 = base + c1_off(s)
                        nc.tensor.matmul(e[pp:pp + 64, o1:o1 + N],
                                         ksb[rs, ti * N + NC0:ti * N + NC0 + 64],
                                         qsb[rs, tn],
                                         tile_position=(32 * s, pp))
                    pt = ptp.tile([C, 1020], BF, tag="pt")
                    e_view = e[:, base:base + 1024].rearrange(
                        "p (b c) -> p b c", b=2)[:, :, 0:510].rearrange(
                        "p b (s c) -> p b s c", s=3)
                    pt_view = pt[:].rearrange("p (b c) -> p b c", b=2).rearrange(
                        "p b (s c) -> p b s c", s=3)
                    nc.scalar.activation(pt_view, e_view, EXP)
                    pts.append(pt)
                    if DBG and t == 0 and u == 0:
                        nc.sync.dma_start(dbg["dbg_pt"][:], pt[:])

                # ---- attn @ v_aug (col-tiled, ones col -> denominator) --
                av = avp.tile([C, 512], DT, tag="av")
                for u in range(2):
                    pt = pts[u]
                    fo = N * u
                    for s in range(4):
                        h = 4 * u + s
                        co = 32 * s
                        o0 = pt_c0(s)
                        nc.tensor.matmul(
                            av[co:co + 32, fo:fo + N],
                            va0[:, 17 * h:17 * h + 32], pt[:, o0:o0 + N],
                            start=True, stop=False, tile_position=(0, co),
                            skip_group_check=True)
                    for s in range(4):
                        h = 4 * u + s
                        co = 32 * s
                        pp = 64 * (s % 2)
                        o1 = pt_c1(s)
                        nc.tensor.matmul(
                            av[co:co + 32, fo:fo + N],
                            va1[pp:pp + NC1, 17 * h:17 * h + 32],
                            pt[pp:pp + NC1, o1:o1 + N],
                            start=False, stop=True, tile_position=(pp, co),
                            skip_group_check=True)

                # ---- evacuate av, accumulate denominators ---------------
                scat = scp.tile([C, 2 * N], BF, tag="scat")
                nc.vector.tensor_copy(scat[:], av[:, 0:2 * N])
                if gi == 0:
                    den_ps = dnb.tile([32, 2 * N], DT, tag="den", name=f"den{gI}")
                    state[("den", gI)] = den_ps
                den_ps = state[("den", gI)]
                nc.tensor.matmul(den_ps[:], gg[:, 32 * gi:32 * gi + 32], scat[:],
                                 start=(gi == 0), stop=(gi == DG - 1),
                                 skip_group_check=True)
                state[("scat", t)] = scat
                if DBG and t == 0:
                    nc.sync.dma_start(dbg["dbg_scat"][:], scat[:])

                # ---- group flush: reciprocal + back half ----------------
                if gi == DG - 1:
                    den_sb = dnp.tile([32, 2 * N], DT, tag="den_sb")
                    nc.vector.tensor_copy(den_sb[:], den_ps[:])
                    rec = dnp.tile([32, 2 * N], DT, tag="rec")
                    nc.vector.reciprocal_approx_fast(rec[:], den_sb[:])
                    recb = dnp.tile([32, 2 * N], BF, tag="recb")
                    nc.vector.tensor_copy(recb[:], rec[:])
                    if DBG and gI == 0:
                        nc.sync.dma_start(dbg["dbg_den"][:], den_sb[:])
                        nc.sync.dma_start(dbg["dbg_rec"][:], rec[:])

                    for tt in range(gI * DG, (gI + 1) * DG):
                        tcJ, tj = divmod(tt, TC)
                        ii = tt % DG
                        rx = scr.tile([C, 512], DT, tag="scr")
                        nc.tensor.matmul(rx[:, 0:2 * N],
                                         selg[:, C * ii:C * ii + C], recb[:])
                        if tj == 0:
                            state[("nrm", tcJ)] = nrm.tile([C, TC * 2 * N], BF,
                                                           tag="normt",
                                                           name=f"nrm{tcJ}")
                        normt = state[("nrm", tcJ)]
                        nc.vector.tensor_tensor(
                            normt[:, tj * 2 * N:(tj + 1) * 2 * N],
                            state[("scat", tt)][:], rx[:, 0:2 * N],
                            mybir.AluOpType.mult)
                        if DBG and tt == 0:
                            rxdbg = nrm.tile([C, 2 * N], DT, tag="rxdbg")
                            nc.vector.tensor_copy(rxdbg[:], rx[:, 0:2 * N])
                            nc.sync.dma_start(dbg["dbg_rx"][:], rxdbg[:])
                            nc.sync.dma_start(dbg["dbg_nrm"][:],
                                              normt[:, 0:2 * N])

                        if tj == TC - 1:
                            fin = scr.tile([C, 512], DT, tag="scr")
                            na = normt[:].rearrange(
                                "p (t g n) -> p t g n", t=TC, g=2)
                            nc.tensor.matmul(fin[:, 0:TC * N], w["woa"][:],
                                             na[:, :, 0, :], start=True,
                                             stop=False)
                            nc.tensor.matmul(fin[:, 0:TC * N], w["wob"][:],
                                             na[:, :, 1, :], start=False,
                                             stop=True)
                            out_sb = osb.tile([C, TC * N], DT, tag="out_sb")
                            nc.vector.tensor_scalar_add(out_sb[:],
                                                        fin[:, 0:TC * N],
                                                        cb[:, 0:1])
                            osl = slice(TC * tcJ, TC * (tcJ + 1))
                            nc.sync.dma_start(
                                out_dram[osl].rearrange("t c n -> c t n"),
                                out_sb[:].rearrange("c (t n) -> c t n", t=TC))

    nc.compile()
    return nc


def _prep_weights(Wv, bv, Wk, bk, Wq, bq, Wo, bo):
    s = F32(1.0 / np.sqrt(C))
    Wq_s = (Wq * s).astype(F32)

    if np.any(bq) or np.any(bk):
        raise NotImplementedError("nonzero q/k biases not folded")

    # head h -> rows 32*(h%4) + 16*(h//4) + d
    wq = np.zeros((C, C), F32)
    wka = np.zeros((C, C), F32)
    wkb = np.zeros((C, C), F32)
    for h in range(H):
        for d in range(D):
            r = 32 * (h % 4) + 16 * (h // 4) + d
            wq[:, r] = Wq_s[D * h + d, :]
            (wka if h < 4 else wkb)[:, r] = Wk[D * h + d, :]

    # output projection: norm rows 32j+r (r<16) hold head j (woa) / head
    # 4+j (wob); rows 32j+16..31 stay zero.
    woa = np.zeros((C, C), F32)
    wob = np.zeros((C, C), F32)
    for j in range(4):
        for r in range(D):
            woa[32 * j + r, :] = Wo[:, D * j + r]
            wob[32 * j + r, :] = Wo[:, D * (4 + j) + r]

    # selector/gather matrices per in-group index i:
    #   gather G_i[C, 32]: den row 4i+j <- scat partition 32j+16
    #   select S_i[32, C]: rx rows 32j..32j+16 <- rec row 4i+j
    selg = np.zeros((32, DG * C), F32)
    gg = np.zeros((C, DG * 32), F32)
    for i in range(DG):
        for j in range(4):
            selg[4 * i + j, C * i + 32 * j:C * i + 32 * j + 17] = 1.0
            gg[32 * j + 16, 32 * i + 4 * i + j] = 1.0

    cb = (bo + Wo @ bv).astype(F32).reshape(C, 1)

    np_bf = mybir.dt.np(BF)
    wm = dict(wq=wq, wka=wka, wkb=wkb,
              wvt=np.ascontiguousarray(Wv.T).astype(F32), woa=woa, wob=wob)
    wm = {k: v.astype(np_bf) for k, v in wm.items()}
    wm["selg"] = selg.astype(np_bf)
    wm["gg"] = gg.astype(np_bf)
    wm["cb"] = cb
    return wm


def kernel(values, keys, query, Wv, bv, Wk, bk, Wq, bq, Wo, bo):
    values = np.asarray(values, F32)
    keys = np.asarray(keys, F32)
    query = np.asarray(query, F32)

    if "nc" not in _cache:
        _cache["nc"] = _build_kernel()
    nc = _cache["nc"]

    wmap = _prep_weights(np.asarray(Wv, F32), np.asarray(bv, F32),
                         np.asarray(Wk, F32), np.asarray(bk, F32),
                         np.asarray(Wq, F32), np.asarray(bq, F32),
                         np.asarray(Wo, F32), np.asarray(bo, F32))

    np_bf = mybir.dt.np(BF)
    qT = np.ascontiguousarray(query.transpose(0, 1, 3, 2)).astype(np_bf)
    kT = np.ascontiguousarray(keys.transpose(0, 1, 3, 2)).astype(np_bf)
    vT = np.ascontiguousarray(values.transpose(0, 1, 3, 2)).astype(np_bf)

    in_maps = [
        dict(q_in=qT[b], k_in=kT[b], v_in=vT[b], **wmap) for b in range(B)
    ]
    results = run_bass_kernel_spmd(nc, in_maps, list(range(B))).results
    out = np.stack([r["out"] for r in results])             # [B, T, C, N]
    return np.ascontiguousarray(out.transpose(0, 1, 3, 2))  # [B, T, N, C]
